# revision 1
# baseline (speedup 1.0000x reference)
"""Trainium2 Bass kernel v2 for the Clements mesh chain (N=512).

Strategy (two-phase, chunk-reassociated):
  Host folds the 1538 primitive layers into 512 2x2-block complex layers
  (G on even pairs, H on odd pairs), then splits the 256 steps into 32
  chunks of 8 steps.

  Phase A (parallel across cores): core c builds the TRANSPOSED chunk
  products W_j.T for j in {c, 8+c, 16+c, 24+c} by running the reversed
  chain on a banded (diagonal-offset) representation with per-partition
  scalar fused ops (tensor_scalar + scalar_tensor_tensor, DVE) in a
  pair-partition layout.  Band half-width <= 17, so per-layer ops are
  [128, ~36] instead of [128, 512].

  Leaves are AllGather'd in 4 rounds of 8 (overlapped with later chunks).

  Phase B (duplicated panel): every core expands each banded leaf to
  dense natural-row lhsT tiles via a skewed DRAM bounce (the skew
  absorbs both the pair->natural row permutation and the diag-offset ->
  absolute-column conversion), then applies the 32 chunk matrices
  sequentially to a 128-column identity panel with PE matmuls
  (fp32r, N=256, ~20 matmuls/chunk).  Cores c and c+4 duplicate the
  same 128-column group; cores 0-3's outputs are used.
"""

import numpy as np

N = 512
S = 256
NCORES = 8
NCH = 32             # chunks
SCH = S // NCH       # 8 steps per chunk
CPC = NCH // NCORES  # 4 chunks per core
LAY = 2 * SCH        # 16 layers emitted per chunk (H.T, G.T per reversed step)
NLAY = CPC * LAY     # 64 layers per core
PB = 2
NSLOT = 16
BW = 40              # banded slots per plane
PADB = 19            # slot of diagonal (pair-base column offset 0)
RS = 416             # skew bounce row stride (>= ~404)
WIN = 384            # dense window per natural K-block (3 full blocks)
PCOLS = 128          # panel columns per core
USE_F32R = True

# dense window starts per K-block b: 3 full M-blocks so every matmul is
# a full M=128 (PE only allows nice dst partition bases; M never affects
# matmul time, which is ~N cycles)
WSTART = [0, 0, 128, 128]


# ----------------------------------------------------------------------------
# Host math (same folding as v1)
# ----------------------------------------------------------------------------

def _mmi_2x2(loss, imb):
    a = np.sqrt(1.0 - loss.astype(np.float64))
    t = a * np.sqrt(0.5 + imb.astype(np.float64))
    r = a * np.sqrt(0.5 - imb.astype(np.float64))
    m = np.zeros(loss.shape + (2, 2), np.complex128)
    m[..., 0, 0] = t
    m[..., 1, 1] = t
    m[..., 0, 1] = 1j * r
    m[..., 1, 0] = 1j * r
    return m


def _pc_vec(theta, loss):
    return np.sqrt(1.0 - loss.astype(np.float64)) * np.exp(1j * theta.astype(np.float64))


def host_fold_layers(inputs):
    th = np.asarray(inputs["thetas_full"], np.float64)
    lp = np.asarray(inputs["pc_losses_full"], np.float64)
    tio = np.asarray(inputs["thetas_inout"], np.float64)
    lio = np.asarray(inputs["pc_losses_inout"], np.float64)
    le = np.asarray(inputs["mmi_losses_even"], np.float64)
    ie = np.asarray(inputs["mmi_imb_even"], np.float64)
    lo = np.asarray(inputs["mmi_losses_odd"], np.float64)
    io = np.asarray(inputs["mmi_imb_odd"], np.float64)

    G = np.zeros((S, 256, 2, 2), np.complex128)
    Hp = np.zeros((S, 255, 2, 2), np.complex128)
    h_edge = np.zeros((S, 2), np.complex128)

    for s in range(S):
        e1 = _mmi_2x2(le[2 * s], ie[2 * s])
        e2 = _mmi_2x2(le[2 * s + 1], ie[2 * s + 1])
        a0 = _pc_vec(th[2 * s], lp[2 * s]).reshape(256, 2)
        G[s] = e2 @ (a0[:, :, None] * e1)

        o1 = _mmi_2x2(lo[2 * s], io[2 * s])
        o2 = _mmi_2x2(lo[2 * s + 1], io[2 * s + 1])
        a1 = _pc_vec(th[2 * s + 1], lp[2 * s + 1])
        a1p = a1[1:-1].reshape(255, 2)
        Hp[s] = o2 @ (a1p[:, :, None] * o1)
        h_edge[s, 0] = a1[0]
        h_edge[s, 1] = a1[-1]

    ain = _pc_vec(tio[0], lio[0]).reshape(256, 2)
    G[0] = G[0] * ain[:, None, :]
    aout = _pc_vec(tio[1], lio[1])
    Hp[S - 1] = aout[1:-1].reshape(255, 2)[:, :, None] * Hp[S - 1]
    h_edge[S - 1, 0] *= aout[0]
    h_edge[S - 1, 1] *= aout[-1]
    return G, Hp, h_edge


def _h_coeffs(Hp_s, edge_s, transpose):
    """One H layer -> (c00, c01, d10, d11) arrays [256] for the E/O update.

    Matrix form: rows 2k+1/2k+2 have block [[h00, h01],[h10, h11]] = Hp[k];
    rows 0 and 511 have scalars edge[0], edge[1].
    Transposed layer swaps h01/h10.
    """
    h00, h01 = Hp_s[:, 0, 0], Hp_s[:, 0, 1]
    h10, h11 = Hp_s[:, 1, 0], Hp_s[:, 1, 1]
    if transpose:
        h01, h10 = h10, h01
    c00 = np.zeros(256, np.complex128)
    c01 = np.zeros(256, np.complex128)
    d10 = np.zeros(256, np.complex128)
    d11 = np.zeros(256, np.complex128)
    c00[:255] = h00
    c00[255] = edge_s[1]
    c01[:255] = h01
    d11[1:] = h11
    d11[0] = edge_s[0]
    d10[1:] = h10
    return c00, c01, d10, d11


def _plane16_G(g):
    """g: [256,2,2] complex -> [256,16] f32 chain scalars."""
    g00, g01 = g[:, 0, 0], g[:, 0, 1]
    g10, g11 = g[:, 1, 0], g[:, 1, 1]
    planes = [
        g00.real, -g00.imag, g01.real, -g01.imag,
        g00.imag, g00.real, g01.imag, g01.real,
        g10.real, -g10.imag, g11.real, -g11.imag,
        g10.imag, g10.real, g11.imag, g11.real,
    ]
    return np.stack(planes, axis=-1).astype(np.float32)


def _plane16_H(c00, c01, d10, d11):
    planes = [
        c00.real, -c00.imag, c01.real, -c01.imag,
        c00.imag, c00.real, c01.imag, c01.real,
        d11.real, -d11.imag, d10.real, -d10.imag,
        d11.imag, d11.real, d10.imag, d10.real,
    ]
    return np.stack(planes, axis=-1).astype(np.float32)


def host_coeff_core(inputs, core, folded=None):
    """Per-core coefficient array [128, NLAY*PB*NSLOT].

    Core c builds W_j.T for chunks j = c, 8+c, 16+c, 24+c.  W_j.T is the
    product of transposed layers in reversed order: for s from high to
    low, apply H_s.T then G_s.T.
    """
    G, Hp, h_edge = folded if folded is not None else host_fold_layers(inputs)
    layers = []   # [256, 16] plane stacks in emission order
    for ch in range(CPC):
        j = 8 * ch + core
        s0, s1 = j * SCH, (j + 1) * SCH
        for s in range(s1 - 1, s0 - 1, -1):
            c00, c01, d10, d11 = _h_coeffs(Hp[s], h_edge[s], transpose=True)
            layers.append(_plane16_H(c00, c01, d10, d11))
            gT = G[s].transpose(0, 2, 1)
            layers.append(_plane16_G(gT))
    arr = np.stack(layers, axis=0)                    # [NLAY, 256, 16]
    arr = arr.reshape(NLAY, PB, 128, NSLOT).transpose(2, 0, 1, 3)
    return np.ascontiguousarray(arr.reshape(128, NLAY * PB * NSLOT))


def host_shift_mats():
    """Constant PE shift matrices [4,128,128]: lhsT[k,m] nonzero -> out[m]=in[k]."""
    m = np.zeros((4, 128, 128), np.float32)
    for i in range(127):
        m[0, i + 1, i] = 1.0      # SH_UP: out[m] = in[m+1]
        m[2, i, i + 1] = 1.0      # SH_DN: out[m] = in[m-1]
    m[1, 0, 127] = 1.0            # SELA: out[127] = in[0]
    m[3, 127, 0] = 1.0            # SELB: out[0] = in[127]
    # SBUF layout [128 partitions, 4*128]: partition = k (contraction)
    return np.ascontiguousarray(m.transpose(1, 0, 2).reshape(128, 512))


def host_panel_init(core):
    """Identity panel [4, 128, 3*PCOLS] (imneg|re|im), natural rows."""
    arr = np.zeros((4, 128, 3 * PCOLS), np.float32)
    colbase = (core % 4) * PCOLS
    for j in range(PCOLS):
        row = colbase + j
        b, p = divmod(row, 128)
        arr[b, p, PCOLS + j] = 1.0   # re plane
    return arr


# ----------------------------------------------------------------------------
# Device program
# ----------------------------------------------------------------------------

def build_program_v2():
    import concourse.bass as bass
    import concourse.tile as tile
    from concourse import bacc, mybir

    f32 = mybir.dt.float32
    f32r = mybir.dt.float32r
    MUL = mybir.AluOpType.mult
    ADD = mybir.AluOpType.add

    nc = bacc.Bacc("TRN2", target_bir_lowering=False, debug=False,
                   num_devices=NCORES)
    coef_d = nc.dram_tensor("coef", [128, NLAY * PB * NSLOT], f32,
                            kind="ExternalInput")
    pinit_d = nc.dram_tensor("pinit", [4, 128, 3 * PCOLS], f32,
                             kind="ExternalInput")
    shm_d = nc.dram_tensor("shmats", [128, 4 * 128], f32, kind="ExternalInput")
    out_d = nc.dram_tensor("mout", [4, 128, 2 * PCOLS], f32,
                           kind="ExternalOutput")

    names = ("E0", "E1", "O0", "O1")

    with tile.TileContext(nc) as tc:
        with (
            tc.tile_pool(name="coefp", bufs=1) as coefp,
            tc.tile_pool(name="leafp", bufs=2) as leafp,
            tc.tile_pool(name="shiftp", bufs=2) as shiftp,
            tc.tile_pool(name="densep", bufs=3) as densep,
            tc.tile_pool(name="panelp", bufs=2) as panelp,
            tc.tile_pool(name="psump", bufs=1, space="PSUM") as psump,
            tc.tile_pool(name="psump2", bufs=2, space="PSUM") as psump2,
            tc.tile_pool(name="dramp", bufs=1, space="DRAM") as dramp,
            tc.tile_pool(name="bouncep", bufs=1, space="DRAM") as bouncep,
        ):
            coef = coefp.tile([128, NLAY * PB * NSLOT], f32)
            nc.sync.dma_start(coef[:], coef_d.ap())
            zeros = coefp.tile([128, RS], f32)
            nc.vector.memset(zeros[:], 0.0)
            shm = coefp.tile([128, 4 * 128], f32)
            nc.sync.dma_start(shm[:], shm_d.ap())
            SH_UP, SH_SELA, SH_DN, SH_SELB = (
                shm[:, 128 * i:128 * (i + 1)] for i in range(4))

            panel = {}
            for b in range(4):
                traw = coefp.tile([128, 3 * PCOLS], f32, name=f"pinit_raw_{b}")
                nc.sync.dma_start(traw[:], pinit_d.ap()[b])
                t = panelp.tile([128, 3 * PCOLS], f32, tag=f"P{b}",
                                name=f"panel_init_{b}")
                nc.vector.tensor_copy(t[:].bitcast(f32r), traw[:])
                panel[b] = t

            def csc(lay, b, slot, p0=0, p1=128):
                idx = (lay * PB + b) * NSLOT + slot
                return coef[p0:p1, idx:idx + 1]

            # ---------- phase A helpers (banded chain) ----------

            def plane(t, pl, lo, hi):
                return t[:, pl * BW + lo: pl * BW + hi]

            # Fixed op range: all chain ops span slots [LO, HI).  Guard
            # slots [0, LO) and [HI, 2) stay permanently zero (memset at
            # chunk start, never written after), so widened shift-copies
            # carry correct zeros and no per-layer memsets are needed.
            LO, HI = 2, BW - 2

            def emit_chains(lay, b, dsts, srcs, base, lo, hi):
                for c in range(len(dsts)):
                    dt_, dpl = dsts[c]
                    d = plane(dt_, dpl, lo, hi)
                    s0 = srcs[0]
                    nc.scalar.mul(d, plane(s0[0], s0[1], lo, hi),
                                  csc(lay, b, base + 4 * c))
                    for k in range(1, 4):
                        sk = srcs[k]
                        off = sk[2] if len(sk) > 2 else 0
                        nc.vector.scalar_tensor_tensor(
                            d, plane(sk[0], sk[1], lo + off, hi + off),
                            csc(lay, b, base + 4 * c + k), d, MUL, ADD)

            lay = 0
            gathered = []
            for ch in range(CPC):
                # fresh leaf state
                cur = {}
                for nm in names:
                    t = leafp.tile([128, 2 * BW], f32, tag=f"L{nm}",
                                   name=f"c{ch}_{nm}")
                    nc.vector.memset(t[:], 0.0)
                    diag = PADB if nm[0] == "E" else PADB + 1
                    nc.vector.memset(t[:, diag:diag + 1], 1.0)
                    cur[nm] = t

                for t_step in range(SCH):
                    fresh = t_step < 1
                    lo_l = max(LO, PADB - 2 * t_step - 4)
                    hi_l = min(HI, PADB + 2 * t_step + 6)
                    # ---- H-type layer ----
                    E0, E1, O0, O1 = (cur[n] for n in names)
                    # shifted operands via PE (shift matrices, fp32):
                    # ep[k] = E[k+1], om[j] = O[j-1]; slot offsets applied
                    # at the consuming AP (-2 for ep, +2 for om).
                    sh = {}
                    for nm in ("ep0", "ep1", "om0", "om1"):
                        sh[nm] = psump.tile([128, 2 * BW], f32, tag=nm,
                                            name=f"sh{ch}_{t_step}_{nm}")
                    nc.tensor.matmul(sh["ep0"][:], SH_UP, E0[:], start=True,
                                     stop=False)
                    nc.tensor.matmul(sh["ep0"][:], SH_SELA, E1[:], start=False,
                                     stop=True)
                    nc.tensor.matmul(sh["ep1"][:], SH_UP, E1[:], start=True,
                                     stop=True)
                    nc.tensor.matmul(sh["om0"][:], SH_DN, O0[:], start=True,
                                     stop=True)
                    nc.tensor.matmul(sh["om1"][:], SH_DN, O1[:], start=True,
                                     stop=False)
                    nc.tensor.matmul(sh["om1"][:], SH_SELB, O0[:], start=False,
                                     stop=True)

                    new = {nm: leafp.tile([128, 2 * BW], f32, tag=f"L{nm}",
                                          name=f"h{ch}_{t_step}_{nm}")
                           for nm in names}
                    for nm in names:
                        nc.gpsimd.memset(new[nm][:], 0.0)
                    for b in range(PB):
                        E, O = (E0, O0) if b == 0 else (E1, O1)
                        nE, nO = (new["E0"], new["O0"]) if b == 0 else (new["E1"], new["O1"])
                        ep = sh["ep0"] if b == 0 else sh["ep1"]
                        om = sh["om0"] if b == 0 else sh["om1"]
                        emit_chains(lay, b, [(nO, 0), (nO, 1)],
                                    [(O, 0), (O, 1), (ep, 0, -2), (ep, 1, -2)],
                                    0, lo_l, hi_l)
                        emit_chains(lay, b, [(nE, 0), (nE, 1)],
                                    [(E, 0), (E, 1), (om, 0, 2), (om, 1, 2)],
                                    8, lo_l, hi_l)
                    cur = new
                    lay += 1

                    # ---- G-type layer ----
                    E0, E1, O0, O1 = (cur[n] for n in names)
                    new = {nm: leafp.tile([128, 2 * BW], f32, tag=f"L{nm}",
                                          name=f"g{ch}_{t_step}_{nm}")
                           for nm in names}
                    for nm in names:
                        nc.gpsimd.memset(new[nm][:], 0.0)
                    for b in range(PB):
                        E, O = (E0, O0) if b == 0 else (E1, O1)
                        nE, nO = (new["E0"], new["O0"]) if b == 0 else (new["E1"], new["O1"])
                        emit_chains(lay, b,
                                    [(nE, 0), (nE, 1), (nO, 0), (nO, 1)],
                                    [(E, 0), (E, 1), (O, 0), (O, 1)], 0,
                                    lo_l, hi_l)
                    cur = new
                    lay += 1

                # round leaves to f32r and send to DRAM + AllGather round
                send = dramp.tile([4, 128, 2 * BW], f32, name=f"send{ch}")
                for i, nm in enumerate(names):
                    rnd = leafp.tile([128, 2 * BW], f32, tag=f"R{nm}",
                                     name=f"r{ch}_{nm}")
                    nc.vector.tensor_copy(rnd[:].bitcast(f32r), cur[nm][:])
                    nc.sync.dma_start(send[i], rnd[:])
                gat = dramp.tile([NCORES, 4, 128, 2 * BW], f32,
                                 name=f"gat{ch}", addr_space="Shared")
                nc.gpsimd.collective_compute(
                    "AllGather", mybir.AluOpType.bypass,
                    replica_groups=[list(range(NCORES))],
                    ins=[send.opt()], outs=[gat.opt()],
                )
                gathered.append(gat)

            # ---------- phase B: expand + panel sweep ----------
            PL = 512 * RS   # plane stride in bounce
            bounces = []
            for bi in range(2):
                bo = bouncep.tile([2 * PL], f32, name=f"bounce{bi}")
                for pl in range(2):
                    for q in range(4):
                        nc.sync.dma_start(
                            bass.AP(bo[:].tensor, pl * PL + q * 128 * RS,
                                    [[RS, 128], [1, RS]]),
                            zeros[:])
                bounces.append(bo)
            for j in range(NCH):
                rnd, owner = j // 8, j % 8
                gat = gathered[rnd]
                bt = bounces[j % 2][:].tensor
                # write banded rows (E rows at c*RS, O rows at c*RS-1; the
                # O guard slots j<3 are zero so the -1 spill is harmless)
                issuers = [nc.sync, nc.scalar]
                for i, nm in enumerate(names):
                    rowbase = {"E0": 0, "E1": 256, "O0": 1, "O1": 257}[nm]
                    odd = -1 if nm[0] == "O" else 0
                    issuers[i % 2].dma_start(
                        bass.AP(bt, rowbase * RS + odd,
                                [[2 * RS, 128], [PL, 2], [1, BW]]),
                        gat[owner, i].rearrange("p (r w) -> p r w", r=2))
                # skew reads into dense natural-row tiles (re|im planes)
                dense = {}
                for bblk in range(4):
                    dt_ = densep.tile([128, 2 * WIN], f32, tag=f"D{bblk}",
                                      name=f"dense{j}_{bblk}")
                    issuers[bblk % 2].dma_start(
                        dt_[:].rearrange("p (r w) -> p r w", r=2).bitcast(f32r),
                        bass.AP(bt, 128 * bblk * (RS - 1) + WSTART[bblk] + PADB,
                                [[RS - 1, 128], [PL, 2], [1, WIN]]).bitcast(f32r))
                    dense[bblk] = dt_

                # panel apply
                newp = {}
                psums = {}
                for m in range(4):
                    ps = psump2.tile([128, 2 * PCOLS], f32, tag=f"ps{m % 2}",
                                     name=f"psum{j}_{m}")
                    psums[m] = ps
                mm_dt = f32r if USE_F32R else f32

                def mm(ps_ap, lhsT_ap, rhs_ap, start, stop=False):
                    nc.tensor.matmul(ps_ap, lhsT_ap.bitcast(mm_dt),
                                     rhs_ap.bitcast(mm_dt),
                                     start=start, stop=stop)

                for m in range(4):
                    mms = []
                    for b_ in (m, m - 1, m + 1):
                        if not 0 <= b_ < 4:
                            continue
                        moff = 128 * m - WSTART[b_]
                        mms.append((dense[b_][:, moff:moff + 128],
                                    panel[b_][:, PCOLS:3 * PCOLS]))
                        mms.append((dense[b_][:, WIN + moff:WIN + moff + 128],
                                    panel[b_][:, 0:2 * PCOLS]))
                    for i_, (lh, rh) in enumerate(mms):
                        mm(psums[m][:], lh, rh, start=(i_ == 0),
                           stop=(i_ == len(mms) - 1))
                for m in range(4):
                    np_ = panelp.tile([128, 3 * PCOLS], f32, tag=f"P{m}",
                                      name=f"panel{j}_{m}")
                    # [re|im] <- psum ; imneg <- -psum_im (both round to f32r)
                    if m % 2 == 0:
                        nc.vector.tensor_copy(
                            np_[:, PCOLS:3 * PCOLS].bitcast(f32r), psums[m][:])
                        nc.scalar.mul(
                            np_[:, 0:PCOLS].bitcast(f32r),
                            psums[m][:, PCOLS:2 * PCOLS], -1.0)
                    else:
                        nc.scalar.mul(
                            np_[:, PCOLS:3 * PCOLS].bitcast(f32r),
                            psums[m][:], 1.0)
                        nc.vector.tensor_scalar_mul(
                            np_[:, 0:PCOLS].bitcast(f32r),
                            psums[m][:, PCOLS:2 * PCOLS], -1.0)
                    newp[m] = np_
                panel = newp

            for b in range(4):
                nc.sync.dma_start(out_d.ap()[b], panel[b][:, PCOLS:3 * PCOLS])

    nc.compile()
    return nc


# ----------------------------------------------------------------------------
# Entry point
# ----------------------------------------------------------------------------

def assemble_output_v2(per_core):
    """per_core: list (cores 0..3 used) of [4, 128, 2*PCOLS] -> [N,N] c64."""
    M = np.zeros((N, N), np.complex64)
    for c in range(4):
        arr = per_core[c]
        cols = slice(c * PCOLS, (c + 1) * PCOLS)
        for b in range(4):
            rows = slice(b * 128, (b + 1) * 128)
            M[rows, cols] = arr[b, :, 0:PCOLS] + 1j * arr[b, :, PCOLS:2 * PCOLS]
    return M


_CACHE = {}


def kernel(**inputs) -> np.ndarray:
    import os

    from concourse.bass_utils import run_bass_kernel_spmd

    folded = host_fold_layers(inputs)
    if "nc" not in _CACHE:
        _CACHE["nc"] = build_program_v2()
    nc = _CACHE["nc"]

    shm = host_shift_mats()
    in_maps = [
        {"coef": host_coeff_core(inputs, c, folded),
         "pinit": host_panel_init(c), "shmats": shm}
        for c in range(NCORES)
    ]
    trace = bool(os.environ.get("KERNEL_TRACE"))
    res = run_bass_kernel_spmd(nc, in_maps, core_ids=list(range(NCORES)),
                               trace=trace)
    if res.exec_time_ns is not None:
        print(f"HW exec time: {res.exec_time_ns} ns")
    return assemble_output_v2([r["mout"] for r in res.results])



# revision 3
# speedup vs baseline: 1.0625x; 1.0625x over previous
"""Trainium2 Bass kernel v2 for the Clements mesh chain (N=512).

Strategy (two-phase, chunk-reassociated):
  Host folds the 1538 primitive layers into 512 2x2-block complex layers
  (G on even pairs, H on odd pairs), then splits the 256 steps into 32
  chunks of 8 steps.

  Phase A (parallel across cores): core c builds the TRANSPOSED chunk
  products W_j.T for j in {c, 8+c, 16+c, 24+c} by running the reversed
  chain on a banded (diagonal-offset) representation with per-partition
  scalar fused ops (tensor_scalar + scalar_tensor_tensor, DVE) in a
  pair-partition layout.  Band half-width <= 17, so per-layer ops are
  [128, ~36] instead of [128, 512].

  Leaves are AllGather'd in 4 rounds of 8 (overlapped with later chunks).

  Phase B (duplicated panel): every core expands each banded leaf to
  dense natural-row lhsT tiles via a skewed DRAM bounce (the skew
  absorbs both the pair->natural row permutation and the diag-offset ->
  absolute-column conversion), then applies the 32 chunk matrices
  sequentially to a 128-column identity panel with PE matmuls
  (fp32r, N=256, ~20 matmuls/chunk).  Cores c and c+4 duplicate the
  same 128-column group; cores 0-3's outputs are used.
"""

import numpy as np

N = 512
S = 256
NCORES = 8
NCH = 32             # chunks
SCH = S // NCH       # 8 steps per chunk
CPC = NCH // NCORES  # 4 chunks per core
LAY = 2 * SCH        # 16 layers emitted per chunk (H.T, G.T per reversed step)
NLAY = CPC * LAY     # 64 layers per core
PB = 2
NSLOT = 16
BW = 40              # banded slots per plane
PADB = 19            # slot of diagonal (pair-base column offset 0)
RS = 416             # skew bounce row stride (>= ~404)
WIN = 384            # dense window per natural K-block (3 full blocks)
PCOLS = 128          # panel columns per core
USE_F32R = True

# dense window starts per K-block b: 3 full M-blocks so every matmul is
# a full M=128 (PE only allows nice dst partition bases; M never affects
# matmul time, which is ~N cycles)
WSTART = [0, 0, 128, 128]


# ----------------------------------------------------------------------------
# Host math (same folding as v1)
# ----------------------------------------------------------------------------

def _mmi_2x2(loss, imb):
    a = np.sqrt(1.0 - loss.astype(np.float64))
    t = a * np.sqrt(0.5 + imb.astype(np.float64))
    r = a * np.sqrt(0.5 - imb.astype(np.float64))
    m = np.zeros(loss.shape + (2, 2), np.complex128)
    m[..., 0, 0] = t
    m[..., 1, 1] = t
    m[..., 0, 1] = 1j * r
    m[..., 1, 0] = 1j * r
    return m


def _pc_vec(theta, loss):
    return np.sqrt(1.0 - loss.astype(np.float64)) * np.exp(1j * theta.astype(np.float64))


def host_fold_layers(inputs):
    th = np.asarray(inputs["thetas_full"], np.float64)
    lp = np.asarray(inputs["pc_losses_full"], np.float64)
    tio = np.asarray(inputs["thetas_inout"], np.float64)
    lio = np.asarray(inputs["pc_losses_inout"], np.float64)
    le = np.asarray(inputs["mmi_losses_even"], np.float64)
    ie = np.asarray(inputs["mmi_imb_even"], np.float64)
    lo = np.asarray(inputs["mmi_losses_odd"], np.float64)
    io = np.asarray(inputs["mmi_imb_odd"], np.float64)

    G = np.zeros((S, 256, 2, 2), np.complex128)
    Hp = np.zeros((S, 255, 2, 2), np.complex128)
    h_edge = np.zeros((S, 2), np.complex128)

    for s in range(S):
        e1 = _mmi_2x2(le[2 * s], ie[2 * s])
        e2 = _mmi_2x2(le[2 * s + 1], ie[2 * s + 1])
        a0 = _pc_vec(th[2 * s], lp[2 * s]).reshape(256, 2)
        G[s] = e2 @ (a0[:, :, None] * e1)

        o1 = _mmi_2x2(lo[2 * s], io[2 * s])
        o2 = _mmi_2x2(lo[2 * s + 1], io[2 * s + 1])
        a1 = _pc_vec(th[2 * s + 1], lp[2 * s + 1])
        a1p = a1[1:-1].reshape(255, 2)
        Hp[s] = o2 @ (a1p[:, :, None] * o1)
        h_edge[s, 0] = a1[0]
        h_edge[s, 1] = a1[-1]

    ain = _pc_vec(tio[0], lio[0]).reshape(256, 2)
    G[0] = G[0] * ain[:, None, :]
    aout = _pc_vec(tio[1], lio[1])
    Hp[S - 1] = aout[1:-1].reshape(255, 2)[:, :, None] * Hp[S - 1]
    h_edge[S - 1, 0] *= aout[0]
    h_edge[S - 1, 1] *= aout[-1]
    return G, Hp, h_edge


def _h_coeffs(Hp_s, edge_s, transpose):
    """One H layer -> (c00, c01, d10, d11) arrays [256] for the E/O update.

    Matrix form: rows 2k+1/2k+2 have block [[h00, h01],[h10, h11]] = Hp[k];
    rows 0 and 511 have scalars edge[0], edge[1].
    Transposed layer swaps h01/h10.
    """
    h00, h01 = Hp_s[:, 0, 0], Hp_s[:, 0, 1]
    h10, h11 = Hp_s[:, 1, 0], Hp_s[:, 1, 1]
    if transpose:
        h01, h10 = h10, h01
    c00 = np.zeros(256, np.complex128)
    c01 = np.zeros(256, np.complex128)
    d10 = np.zeros(256, np.complex128)
    d11 = np.zeros(256, np.complex128)
    c00[:255] = h00
    c00[255] = edge_s[1]
    c01[:255] = h01
    d11[1:] = h11
    d11[0] = edge_s[0]
    d10[1:] = h10
    return c00, c01, d10, d11


def _plane16_G(g):
    """g: [256,2,2] complex -> [256,16] f32 chain scalars."""
    g00, g01 = g[:, 0, 0], g[:, 0, 1]
    g10, g11 = g[:, 1, 0], g[:, 1, 1]
    planes = [
        g00.real, -g00.imag, g01.real, -g01.imag,
        g00.imag, g00.real, g01.imag, g01.real,
        g10.real, -g10.imag, g11.real, -g11.imag,
        g10.imag, g10.real, g11.imag, g11.real,
    ]
    return np.stack(planes, axis=-1).astype(np.float32)


def _plane16_H(c00, c01, d10, d11):
    planes = [
        c00.real, -c00.imag, c01.real, -c01.imag,
        c00.imag, c00.real, c01.imag, c01.real,
        d11.real, -d11.imag, d10.real, -d10.imag,
        d11.imag, d11.real, d10.imag, d10.real,
    ]
    return np.stack(planes, axis=-1).astype(np.float32)


def host_coeff_core(inputs, core, folded=None):
    """Per-core coefficient array [128, NLAY*PB*NSLOT].

    Core c builds W_j.T for chunks j = c, 8+c, 16+c, 24+c.  W_j.T is the
    product of transposed layers in reversed order: for s from high to
    low, apply H_s.T then G_s.T.
    """
    G, Hp, h_edge = folded if folded is not None else host_fold_layers(inputs)
    layers = []   # [256, 16] plane stacks in emission order
    for ch in range(CPC):
        j = 8 * ch + core
        s0, s1 = j * SCH, (j + 1) * SCH
        for s in range(s1 - 1, s0 - 1, -1):
            c00, c01, d10, d11 = _h_coeffs(Hp[s], h_edge[s], transpose=True)
            layers.append(_plane16_H(c00, c01, d10, d11))
            gT = G[s].transpose(0, 2, 1)
            layers.append(_plane16_G(gT))
    arr = np.stack(layers, axis=0)                    # [NLAY, 256, 16]
    arr = arr.reshape(NLAY, PB, 128, NSLOT).transpose(2, 0, 1, 3)
    return np.ascontiguousarray(arr.reshape(128, NLAY * PB * NSLOT))


def host_shift_mats():
    """Constant PE shift matrices [4,128,128]: lhsT[k,m] nonzero -> out[m]=in[k]."""
    m = np.zeros((4, 128, 128), np.float32)
    for i in range(127):
        m[0, i + 1, i] = 1.0      # SH_UP: out[m] = in[m+1]
        m[2, i, i + 1] = 1.0      # SH_DN: out[m] = in[m-1]
    m[1, 0, 127] = 1.0            # SELA: out[127] = in[0]
    m[3, 127, 0] = 1.0            # SELB: out[0] = in[127]
    # SBUF layout [128 partitions, 4*128]: partition = k (contraction)
    return np.ascontiguousarray(m.transpose(1, 0, 2).reshape(128, 512))


def host_panel_init(core):
    """Identity panel [4, 128, 3*PCOLS] (imneg|re|im), natural rows."""
    arr = np.zeros((4, 128, 3 * PCOLS), np.float32)
    colbase = (core % 4) * PCOLS
    for j in range(PCOLS):
        row = colbase + j
        b, p = divmod(row, 128)
        arr[b, p, PCOLS + j] = 1.0   # re plane
    return arr


# ----------------------------------------------------------------------------
# Device program
# ----------------------------------------------------------------------------

def build_program_v2():
    import concourse.bass as bass
    import concourse.tile as tile
    from concourse import bacc, mybir

    f32 = mybir.dt.float32
    f32r = mybir.dt.float32r
    MUL = mybir.AluOpType.mult
    ADD = mybir.AluOpType.add

    nc = bacc.Bacc("TRN2", target_bir_lowering=False, debug=False,
                   num_devices=NCORES)
    coef_d = nc.dram_tensor("coef", [128, NLAY * PB * NSLOT], f32,
                            kind="ExternalInput")
    pinit_d = nc.dram_tensor("pinit", [4, 128, 3 * PCOLS], f32,
                             kind="ExternalInput")
    shm_d = nc.dram_tensor("shmats", [128, 4 * 128], f32, kind="ExternalInput")
    out_d = nc.dram_tensor("mout", [4, 128, 2 * PCOLS], f32,
                           kind="ExternalOutput")

    names = ("E0", "E1", "O0", "O1")

    with tile.TileContext(nc) as tc:
        with (
            tc.tile_pool(name="coefp", bufs=1) as coefp,
            tc.tile_pool(name="leafp", bufs=2) as leafp,
            tc.tile_pool(name="shiftp", bufs=2) as shiftp,
            tc.tile_pool(name="densep", bufs=3) as densep,
            tc.tile_pool(name="panelp", bufs=2) as panelp,
            tc.tile_pool(name="psump", bufs=1, space="PSUM") as psump,
            tc.tile_pool(name="psump2", bufs=2, space="PSUM") as psump2,
            tc.tile_pool(name="dramp", bufs=1, space="DRAM") as dramp,
            tc.tile_pool(name="bouncep", bufs=1, space="DRAM") as bouncep,
        ):
            coef = coefp.tile([128, NLAY * PB * NSLOT], f32)
            nc.sync.dma_start(coef[:], coef_d.ap())
            zeros = coefp.tile([128, RS], f32)
            nc.vector.memset(zeros[:], 0.0)
            shm = coefp.tile([128, 4 * 128], f32)
            nc.sync.dma_start(shm[:], shm_d.ap())
            SH_UP, SH_SELA, SH_DN, SH_SELB = (
                shm[:, 128 * i:128 * (i + 1)] for i in range(4))

            panel = {}
            for b in range(4):
                traw = coefp.tile([128, 3 * PCOLS], f32, name=f"pinit_raw_{b}")
                nc.sync.dma_start(traw[:], pinit_d.ap()[b])
                t = panelp.tile([128, 3 * PCOLS], f32, tag=f"P{b}",
                                name=f"panel_init_{b}")
                nc.vector.tensor_copy(t[:].bitcast(f32r), traw[:])
                panel[b] = t

            def csc(lay, b, slot, p0=0, p1=128):
                idx = (lay * PB + b) * NSLOT + slot
                return coef[p0:p1, idx:idx + 1]

            # ---------- phase A helpers (banded chain) ----------

            def plane(t, pl, lo, hi):
                return t[:, pl * BW + lo: pl * BW + hi]

            # Fixed op range: all chain ops span slots [LO, HI).  Guard
            # slots [0, LO) and [HI, 2) stay permanently zero (memset at
            # chunk start, never written after), so widened shift-copies
            # carry correct zeros and no per-layer memsets are needed.
            LO, HI = 2, BW - 2

            def emit_chains(lay, b, dsts, srcs, base, lo, hi):
                # First op: scalar.mul over the FULL plane [0, BW) from an
                # unshifted source whose guard cols are zero -> dst guards
                # get written to zero, so no per-layer memset is needed.
                for c in range(len(dsts)):
                    dt_, dpl = dsts[c]
                    d = plane(dt_, dpl, lo, hi)
                    s0 = srcs[0]
                    nc.scalar.mul(plane(dt_, dpl, 0, BW),
                                  plane(s0[0], s0[1], 0, BW),
                                  csc(lay, b, base + 4 * c))
                    for k in range(1, 4):
                        sk = srcs[k]
                        off = sk[2] if len(sk) > 2 else 0
                        nc.vector.scalar_tensor_tensor(
                            d, plane(sk[0], sk[1], lo + off, hi + off),
                            csc(lay, b, base + 4 * c + k), d, MUL, ADD)

            # ---------- phase B plumbing (emitted interleaved) ----------
            PL = 512 * RS   # plane stride in bounce
            bounces = []
            for bi in range(2):
                bo = bouncep.tile([2 * PL], f32, name=f"bounce{bi}")
                for pl in range(2):
                    for q in range(4):
                        nc.sync.dma_start(
                            bass.AP(bo[:].tensor, pl * PL + q * 128 * RS,
                                    [[RS, 128], [1, RS]]),
                            zeros[:])
                bounces.append(bo)

            gathered = []
            panel_box = {"panel": panel}
            mm_dt = f32r if USE_F32R else f32

            def emit_phaseB_chunk(j):
                panel = panel_box["panel"]
                rnd_i, owner = j // 8, j % 8
                gat = gathered[rnd_i]
                bt = bounces[j % 2][:].tensor
                # write banded rows (E rows at c*RS, O rows at c*RS-1; the
                # O guard slots j<3 are zero so the -1 spill is harmless).
                # Bounce writes go on the gpsimd queue: they depend on the
                # AllGather anyway, and gpsimd's FIFO is blocked by the
                # collective trigger until it completes.
                for i, nm in enumerate(names):
                    rowbase = {"E0": 0, "E1": 256, "O0": 1, "O1": 257}[nm]
                    odd = -1 if nm[0] == "O" else 0
                    nc.gpsimd.dma_start(
                        bass.AP(bt, rowbase * RS + odd,
                                [[2 * RS, 128], [PL, 2], [1, BW]]),
                        gat[owner, i].rearrange("p (r w) -> p r w", r=2))
                # skew reads into dense natural-row tiles (re|im planes)
                dense = {}
                for bblk in range(4):
                    dt_ = densep.tile([128, 2 * WIN], f32, tag=f"D{bblk}",
                                      name=f"dense{j}_{bblk}")
                    nc.sync.dma_start(
                        dt_[:].rearrange("p (r w) -> p r w", r=2).bitcast(f32r),
                        bass.AP(bt, 128 * bblk * (RS - 1) + WSTART[bblk] + PADB,
                                [[RS - 1, 128], [PL, 2], [1, WIN]]).bitcast(f32r))
                    dense[bblk] = dt_

                # panel apply
                psums = {}
                for m in range(4):
                    ps = psump2.tile([128, 2 * PCOLS], f32, tag=f"ps{m % 2}",
                                     name=f"psum{j}_{m}")
                    psums[m] = ps

                def mm(ps_ap, lhsT_ap, rhs_ap, start, stop=False):
                    nc.tensor.matmul(ps_ap, lhsT_ap.bitcast(mm_dt),
                                     rhs_ap.bitcast(mm_dt),
                                     start=start, stop=stop)

                for m in range(4):
                    mms = []
                    for b_ in (m, m - 1, m + 1):
                        if not 0 <= b_ < 4:
                            continue
                        moff = 128 * m - WSTART[b_]
                        mms.append((dense[b_][:, moff:moff + 128],
                                    panel[b_][:, PCOLS:3 * PCOLS]))
                        mms.append((dense[b_][:, WIN + moff:WIN + moff + 128],
                                    panel[b_][:, 0:2 * PCOLS]))
                    for i_, (lh, rh) in enumerate(mms):
                        mm(psums[m][:], lh, rh, start=(i_ == 0),
                           stop=(i_ == len(mms) - 1))
                newp = {}
                for m in range(4):
                    np_ = panelp.tile([128, 3 * PCOLS], f32, tag=f"P{m}",
                                      name=f"panel{j}_{m}")
                    # [re|im] <- psum ; imneg <- -psum_im (both round to f32r)
                    if m % 2 == 0:
                        nc.vector.tensor_copy(
                            np_[:, PCOLS:3 * PCOLS].bitcast(f32r), psums[m][:])
                        nc.scalar.mul(
                            np_[:, 0:PCOLS].bitcast(f32r),
                            psums[m][:, PCOLS:2 * PCOLS], -1.0)
                    else:
                        nc.scalar.mul(
                            np_[:, PCOLS:3 * PCOLS].bitcast(f32r),
                            psums[m][:], 1.0)
                        nc.vector.tensor_scalar_mul(
                            np_[:, 0:PCOLS].bitcast(f32r),
                            psums[m][:, PCOLS:2 * PCOLS], -1.0)
                    newp[m] = np_
                panel_box["panel"] = newp

            # ---------- phase A chunks with interleaved phase B ----------
            lay = 0
            for ch in range(CPC):
                # fresh leaf state
                cur = {}
                for nm in names:
                    t = leafp.tile([128, 2 * BW], f32, tag=f"L{nm}",
                                   name=f"c{ch}_{nm}")
                    nc.vector.memset(t[:], 0.0)
                    diag = PADB if nm[0] == "E" else PADB + 1
                    nc.vector.memset(t[:, diag:diag + 1], 1.0)
                    cur[nm] = t

                for t_step in range(SCH):
                    lo_l = max(LO, PADB - 2 * t_step - 4)
                    hi_l = min(HI, PADB + 2 * t_step + 6)
                    # ---- H-type layer ----
                    E0, E1, O0, O1 = (cur[n] for n in names)
                    # shifted operands via PE (shift matrices, fp32):
                    # ep[k] = E[k+1], om[j] = O[j-1]; slot offsets applied
                    # at the consuming AP (-2 for ep, +2 for om).
                    sh = {}
                    for nm in ("ep0", "ep1", "om0", "om1"):
                        sh[nm] = psump.tile([128, 2 * BW], f32, tag=nm,
                                            name=f"sh{ch}_{t_step}_{nm}")
                    nc.tensor.matmul(sh["ep0"][:], SH_UP, E0[:], start=True,
                                     stop=False)
                    nc.tensor.matmul(sh["ep0"][:], SH_SELA, E1[:], start=False,
                                     stop=True)
                    nc.tensor.matmul(sh["ep1"][:], SH_UP, E1[:], start=True,
                                     stop=True)
                    nc.tensor.matmul(sh["om0"][:], SH_DN, O0[:], start=True,
                                     stop=True)
                    nc.tensor.matmul(sh["om1"][:], SH_DN, O1[:], start=True,
                                     stop=False)
                    nc.tensor.matmul(sh["om1"][:], SH_SELB, O0[:], start=False,
                                     stop=True)

                    new = {nm: leafp.tile([128, 2 * BW], f32, tag=f"L{nm}",
                                          name=f"h{ch}_{t_step}_{nm}")
                           for nm in names}
                    for b in range(PB):
                        E, O = (E0, O0) if b == 0 else (E1, O1)
                        nE, nO = (new["E0"], new["O0"]) if b == 0 else (new["E1"], new["O1"])
                        ep = sh["ep0"] if b == 0 else sh["ep1"]
                        om = sh["om0"] if b == 0 else sh["om1"]
                        emit_chains(lay, b, [(nO, 0), (nO, 1)],
                                    [(O, 0), (O, 1), (ep, 0, -2), (ep, 1, -2)],
                                    0, lo_l, hi_l)
                        emit_chains(lay, b, [(nE, 0), (nE, 1)],
                                    [(E, 0), (E, 1), (om, 0, 2), (om, 1, 2)],
                                    8, lo_l, hi_l)
                    cur = new
                    lay += 1

                    # ---- G-type layer ----
                    E0, E1, O0, O1 = (cur[n] for n in names)
                    new = {nm: leafp.tile([128, 2 * BW], f32, tag=f"L{nm}",
                                          name=f"g{ch}_{t_step}_{nm}")
                           for nm in names}
                    for b in range(PB):
                        E, O = (E0, O0) if b == 0 else (E1, O1)
                        nE, nO = (new["E0"], new["O0"]) if b == 0 else (new["E1"], new["O1"])
                        emit_chains(lay, b,
                                    [(nE, 0), (nE, 1), (nO, 0), (nO, 1)],
                                    [(E, 0), (E, 1), (O, 0), (O, 1)], 0,
                                    lo_l, hi_l)
                    cur = new
                    lay += 1

                    # interleave one phase-B chunk of the previous round per
                    # phase-A step, so every engine FIFO alternates A/B work
                    # at fine grain and the collective overlaps compute.
                    if ch >= 1:
                        emit_phaseB_chunk(8 * (ch - 1) + t_step)

                # round leaves to f32r and send to DRAM + AllGather round
                send = dramp.tile([4, 128, 2 * BW], f32, name=f"send{ch}",
                                  tag=f"send{ch}")
                for i, nm in enumerate(names):
                    rnd = leafp.tile([128, 2 * BW], f32, tag=f"R{nm}",
                                     name=f"r{ch}_{nm}")
                    nc.vector.tensor_copy(rnd[:].bitcast(f32r), cur[nm][:])
                    nc.sync.dma_start(send[i], rnd[:])
                gat = dramp.tile([NCORES, 4, 128, 2 * BW], f32,
                                 name=f"gat{ch}", tag=f"gat{ch}",
                                 addr_space="Shared")
                nc.gpsimd.collective_compute(
                    "AllGather", mybir.AluOpType.bypass,
                    replica_groups=[list(range(NCORES))],
                    ins=[send.opt()], outs=[gat.opt()],
                )
                gathered.append(gat)

            # last round of phase B after all phase A work
            for t_step in range(SCH):
                emit_phaseB_chunk(8 * (CPC - 1) + t_step)

            panel = panel_box["panel"]
            for b in range(4):
                nc.sync.dma_start(out_d.ap()[b], panel[b][:, PCOLS:3 * PCOLS])

    nc.compile()
    return nc


# ----------------------------------------------------------------------------
# Entry point
# ----------------------------------------------------------------------------

def assemble_output_v2(per_core):
    """per_core: list (cores 0..3 used) of [4, 128, 2*PCOLS] -> [N,N] c64."""
    M = np.zeros((N, N), np.complex64)
    for c in range(4):
        arr = per_core[c]
        cols = slice(c * PCOLS, (c + 1) * PCOLS)
        for b in range(4):
            rows = slice(b * 128, (b + 1) * 128)
            M[rows, cols] = arr[b, :, 0:PCOLS] + 1j * arr[b, :, PCOLS:2 * PCOLS]
    return M


_CACHE = {}


def kernel(**inputs) -> np.ndarray:
    import os

    from concourse.bass_utils import run_bass_kernel_spmd

    folded = host_fold_layers(inputs)
    if "nc" not in _CACHE:
        _CACHE["nc"] = build_program_v2()
    nc = _CACHE["nc"]

    shm = host_shift_mats()
    in_maps = [
        {"coef": host_coeff_core(inputs, c, folded),
         "pinit": host_panel_init(c), "shmats": shm}
        for c in range(NCORES)
    ]
    trace = bool(os.environ.get("KERNEL_TRACE"))
    res = run_bass_kernel_spmd(nc, in_maps, core_ids=list(range(NCORES)),
                               trace=trace)
    if res.exec_time_ns is not None:
        print(f"HW exec time: {res.exec_time_ns} ns")
    return assemble_output_v2([r["mout"] for r in res.results])



# revision 8
# speedup vs baseline: 1.0651x; 1.0024x over previous
"""Trainium2 Bass kernel v2 for the Clements mesh chain (N=512).

Strategy (two-phase, chunk-reassociated):
  Host folds the 1538 primitive layers into 512 2x2-block complex layers
  (G on even pairs, H on odd pairs), then splits the 256 steps into 32
  chunks of 8 steps.

  Phase A (parallel across cores): core c builds the TRANSPOSED chunk
  products W_j.T for j in {c, 8+c, 16+c, 24+c} by running the reversed
  chain on a banded (diagonal-offset) representation with per-partition
  scalar fused ops (tensor_scalar + scalar_tensor_tensor, DVE) in a
  pair-partition layout.  Band half-width <= 17, so per-layer ops are
  [128, ~36] instead of [128, 512].

  Leaves are AllGather'd in 4 rounds of 8 (overlapped with later chunks).

  Phase B (duplicated panel): every core expands each banded leaf to
  dense natural-row lhsT tiles via a skewed DRAM bounce (the skew
  absorbs both the pair->natural row permutation and the diag-offset ->
  absolute-column conversion), then applies the 32 chunk matrices
  sequentially to a 128-column identity panel with PE matmuls
  (fp32r, N=256, ~20 matmuls/chunk).  Cores c and c+4 duplicate the
  same 128-column group; cores 0-3's outputs are used.
"""

import numpy as np

N = 512
S = 256
NCORES = 8
NCH = 32             # chunks
SCH = S // NCH       # 8 steps per chunk
CPC = NCH // NCORES  # 4 chunks per core
LAY = 2 * SCH        # 16 layers emitted per chunk (H.T, G.T per reversed step)
NLAY = CPC * LAY     # 64 layers per core
PB = 2
NSLOT = 8
BW = 40              # banded slots per plane
TW = 3 * BW + 4      # state tile width: [pad2 | imneg | re | im | pad2]
PADB = 19            # slot of diagonal (pair-base column offset 0)
RS = 416             # skew bounce row stride (>= ~404)
WIN = 384            # dense window per natural K-block (3 full blocks)
PCOLS = 128          # panel columns per core
USE_F32R = True

# dense window starts per K-block b: 3 full M-blocks so every matmul is
# a full M=128 (PE only allows nice dst partition bases; M never affects
# matmul time, which is ~N cycles)
WSTART = [0, 0, 128, 128]


# ----------------------------------------------------------------------------
# Host math (same folding as v1)
# ----------------------------------------------------------------------------

def _mmi_2x2(loss, imb):
    a = np.sqrt(1.0 - loss.astype(np.float64))
    t = a * np.sqrt(0.5 + imb.astype(np.float64))
    r = a * np.sqrt(0.5 - imb.astype(np.float64))
    m = np.zeros(loss.shape + (2, 2), np.complex128)
    m[..., 0, 0] = t
    m[..., 1, 1] = t
    m[..., 0, 1] = 1j * r
    m[..., 1, 0] = 1j * r
    return m


def _pc_vec(theta, loss):
    return np.sqrt(1.0 - loss.astype(np.float64)) * np.exp(1j * theta.astype(np.float64))


def host_fold_layers(inputs):
    th = np.asarray(inputs["thetas_full"], np.float64)
    lp = np.asarray(inputs["pc_losses_full"], np.float64)
    tio = np.asarray(inputs["thetas_inout"], np.float64)
    lio = np.asarray(inputs["pc_losses_inout"], np.float64)
    le = np.asarray(inputs["mmi_losses_even"], np.float64)
    ie = np.asarray(inputs["mmi_imb_even"], np.float64)
    lo = np.asarray(inputs["mmi_losses_odd"], np.float64)
    io = np.asarray(inputs["mmi_imb_odd"], np.float64)

    G = np.zeros((S, 256, 2, 2), np.complex128)
    Hp = np.zeros((S, 255, 2, 2), np.complex128)
    h_edge = np.zeros((S, 2), np.complex128)

    for s in range(S):
        e1 = _mmi_2x2(le[2 * s], ie[2 * s])
        e2 = _mmi_2x2(le[2 * s + 1], ie[2 * s + 1])
        a0 = _pc_vec(th[2 * s], lp[2 * s]).reshape(256, 2)
        G[s] = e2 @ (a0[:, :, None] * e1)

        o1 = _mmi_2x2(lo[2 * s], io[2 * s])
        o2 = _mmi_2x2(lo[2 * s + 1], io[2 * s + 1])
        a1 = _pc_vec(th[2 * s + 1], lp[2 * s + 1])
        a1p = a1[1:-1].reshape(255, 2)
        Hp[s] = o2 @ (a1p[:, :, None] * o1)
        h_edge[s, 0] = a1[0]
        h_edge[s, 1] = a1[-1]

    ain = _pc_vec(tio[0], lio[0]).reshape(256, 2)
    G[0] = G[0] * ain[:, None, :]
    aout = _pc_vec(tio[1], lio[1])
    Hp[S - 1] = aout[1:-1].reshape(255, 2)[:, :, None] * Hp[S - 1]
    h_edge[S - 1, 0] *= aout[0]
    h_edge[S - 1, 1] *= aout[-1]
    return G, Hp, h_edge


def _h_coeffs(Hp_s, edge_s, transpose):
    """One H layer -> (c00, c01, d10, d11) arrays [256] for the E/O update.

    Matrix form: rows 2k+1/2k+2 have block [[h00, h01],[h10, h11]] = Hp[k];
    rows 0 and 511 have scalars edge[0], edge[1].
    Transposed layer swaps h01/h10.
    """
    h00, h01 = Hp_s[:, 0, 0], Hp_s[:, 0, 1]
    h10, h11 = Hp_s[:, 1, 0], Hp_s[:, 1, 1]
    if transpose:
        h01, h10 = h10, h01
    c00 = np.zeros(256, np.complex128)
    c01 = np.zeros(256, np.complex128)
    d10 = np.zeros(256, np.complex128)
    d11 = np.zeros(256, np.complex128)
    c00[:255] = h00
    c00[255] = edge_s[1]
    c01[:255] = h01
    d11[1:] = h11
    d11[0] = edge_s[0]
    d10[1:] = h10
    return c00, c01, d10, d11


def _plane8_G(g):
    """g: [256,2,2] complex -> [256,8] f32 chain scalars (imneg scheme).

    dst nE = g00*E + g01*O ; dst nO = g10*E + g11*O.  Each complex coeff c
    contributes two per-partition scalars: c.real on view [re|im] and
    c.imag on view [-im|re].
    """
    g00, g01 = g[:, 0, 0], g[:, 0, 1]
    g10, g11 = g[:, 1, 0], g[:, 1, 1]
    planes = [
        g00.real, g00.imag, g01.real, g01.imag,
        g10.real, g10.imag, g11.real, g11.imag,
    ]
    return np.stack(planes, axis=-1).astype(np.float32)


def _plane8_H(c00, c01, d10, d11):
    """dst nO = c00*O + c01*ep ; dst nE = d11*E + d10*om."""
    planes = [
        c00.real, c00.imag, c01.real, c01.imag,
        d11.real, d11.imag, d10.real, d10.imag,
    ]
    return np.stack(planes, axis=-1).astype(np.float32)


def host_coeff_core(inputs, core, folded=None):
    """Per-core coefficient array [128, NLAY*PB*NSLOT].

    Core c builds W_j.T for chunks j = c, 8+c, 16+c, 24+c.  W_j.T is the
    product of transposed layers in reversed order: for s from high to
    low, apply H_s.T then G_s.T.
    """
    G, Hp, h_edge = folded if folded is not None else host_fold_layers(inputs)
    layers = []   # [256, 16] plane stacks in emission order
    for ch in range(CPC):
        j = 8 * ch + core
        s0, s1 = j * SCH, (j + 1) * SCH
        for s in range(s1 - 1, s0 - 1, -1):
            c00, c01, d10, d11 = _h_coeffs(Hp[s], h_edge[s], transpose=True)
            layers.append(_plane8_H(c00, c01, d10, d11))
            gT = G[s].transpose(0, 2, 1)
            layers.append(_plane8_G(gT))
    arr = np.stack(layers, axis=0)                    # [NLAY, 256, 8]
    arr = arr.reshape(NLAY, PB, 128, NSLOT).transpose(2, 0, 1, 3)
    return np.ascontiguousarray(arr.reshape(128, NLAY * PB * NSLOT))


def host_shift_mats():
    """Constant PE shift matrices [4,128,128]: lhsT[k,m] nonzero -> out[m]=in[k]."""
    m = np.zeros((4, 128, 128), np.float32)
    for i in range(127):
        m[0, i + 1, i] = 1.0      # SH_UP: out[m] = in[m+1]
        m[2, i, i + 1] = 1.0      # SH_DN: out[m] = in[m-1]
    m[1, 0, 127] = 1.0            # SELA: out[127] = in[0]
    m[3, 127, 0] = 1.0            # SELB: out[0] = in[127]
    # SBUF layout [128 partitions, 4*128]: partition = k (contraction)
    return np.ascontiguousarray(m.transpose(1, 0, 2).reshape(128, 512))


def host_panel_init(core):
    """Identity panel [4, 128, 3*PCOLS] (imneg|re|im), natural rows."""
    arr = np.zeros((4, 128, 3 * PCOLS), np.float32)
    colbase = (core % 4) * PCOLS
    for j in range(PCOLS):
        row = colbase + j
        b, p = divmod(row, 128)
        arr[b, p, PCOLS + j] = 1.0   # re plane
    return arr


# ----------------------------------------------------------------------------
# Device program
# ----------------------------------------------------------------------------

def build_program_v2():
    import concourse.bass as bass
    import concourse.tile as tile
    from concourse import bacc, mybir

    f32 = mybir.dt.float32
    f32r = mybir.dt.float32r
    MUL = mybir.AluOpType.mult
    ADD = mybir.AluOpType.add

    nc = bacc.Bacc("TRN2", target_bir_lowering=False, debug=False,
                   num_devices=NCORES)
    coef_d = nc.dram_tensor("coef", [128, NLAY * PB * NSLOT], f32,
                            kind="ExternalInput")
    pinit_d = nc.dram_tensor("pinit", [4, 128, 3 * PCOLS], f32,
                             kind="ExternalInput")
    shm_d = nc.dram_tensor("shmats", [128, 4 * 128], f32, kind="ExternalInput")
    out_d = nc.dram_tensor("mout", [4, 128, 2 * PCOLS], f32,
                           kind="ExternalOutput")

    names = ("E0", "E1", "O0", "O1")

    with tile.TileContext(nc) as tc:
        with (
            tc.tile_pool(name="coefp", bufs=1) as coefp,
            tc.tile_pool(name="leafp", bufs=2) as leafp,
            tc.tile_pool(name="shiftp", bufs=2) as shiftp,
            tc.tile_pool(name="densep", bufs=3) as densep,
            tc.tile_pool(name="panelp", bufs=2) as panelp,
            tc.tile_pool(name="psump", bufs=1, space="PSUM") as psump,
            tc.tile_pool(name="psump2", bufs=2, space="PSUM") as psump2,
            tc.tile_pool(name="dramp", bufs=1, space="DRAM") as dramp,
            tc.tile_pool(name="bouncep", bufs=1, space="DRAM") as bouncep,
        ):
            coef = coefp.tile([128, NLAY * PB * NSLOT], f32)
            nc.sync.dma_start(coef[:], coef_d.ap())
            zeros = coefp.tile([128, RS], f32)
            nc.vector.memset(zeros[:], 0.0)
            shm = coefp.tile([128, 4 * 128], f32)
            nc.sync.dma_start(shm[:], shm_d.ap())
            SH_UP, SH_SELA, SH_DN, SH_SELB = (
                shm[:, 128 * i:128 * (i + 1)] for i in range(4))

            panel = {}
            for b in range(4):
                traw = coefp.tile([128, 3 * PCOLS], f32, name=f"pinit_raw_{b}")
                nc.sync.dma_start(traw[:], pinit_d.ap()[b])
                t = panelp.tile([128, 3 * PCOLS], f32, tag=f"P{b}",
                                name=f"panel_init_{b}")
                nc.vector.tensor_copy(t[:].bitcast(f32r), traw[:])
                panel[b] = t

            def csc(lay, b, slot, p0=0, p1=128):
                idx = (lay * PB + b) * NSLOT + slot
                return coef[p0:p1, idx:idx + 1]

            # ---------- phase A helpers (banded chain, imneg scheme) ----------
            # State tiles [128, TW]: [pad2 | imneg(BW) | re(BW) | im(BW) | pad2].
            # A complex MAC  dst += c*src  is two per-partition-scalar ops:
            #   dst[re|im] += c.real * src[re|im]  +  c.imag * src[-im|re]

            LO, HI = 2, BW - 2

            def p3(t):
                return t[:, 2:2 + 3 * BW].rearrange("p (a w) -> p a w", a=3)

            def v1(t, lo, hi, off=0):
                return p3(t)[:, 1:3, lo + off:hi + off]     # [re|im]

            def v2(t, lo, hi, off=0):
                return p3(t)[:, 0:2, lo + off:hi + off]     # [-im|re]

            def emit_dst(lay, b, nt, s1t, s2t, base, lo, hi, off2=0):
                """nt = c1*s1t + c2*s2t (complex per-partition coeffs) plus
                imneg-plane maintenance.  s2t may be a shifted PSUM tile
                consumed at slot offset off2.  The first op spans the full
                [re|im] planes so guard cols propagate zeros."""
                full = nt[:, 2 + BW:2 + 3 * BW]
                nc.scalar.mul(full, s1t[:, 2 + BW:2 + 3 * BW],
                              csc(lay, b, base + 0))
                d = v1(nt, lo, hi)
                nc.vector.scalar_tensor_tensor(
                    d, v2(s1t, lo, hi), csc(lay, b, base + 1), d, MUL, ADD)
                nc.vector.scalar_tensor_tensor(
                    d, v1(s2t, lo, hi, off2), csc(lay, b, base + 2), d, MUL, ADD)
                nc.vector.scalar_tensor_tensor(
                    d, v2(s2t, lo, hi, off2), csc(lay, b, base + 3), d, MUL, ADD)
                # imneg plane <- -im
                nc.scalar.mul(nt[:, 2:2 + BW],
                              nt[:, 2 + 2 * BW:2 + 3 * BW], -1.0)

            # ---------- phase B plumbing (emitted interleaved) ----------
            PL = 512 * RS   # plane stride in bounce
            bounces = []
            for bi in range(2):
                bo = bouncep.tile([2 * PL], f32, name=f"bounce{bi}")
                for pl in range(2):
                    for q in range(4):
                        nc.sync.dma_start(
                            bass.AP(bo[:].tensor, pl * PL + q * 128 * RS,
                                    [[RS, 128], [1, RS]]),
                            zeros[:])
                bounces.append(bo)

            gathered = []
            panel_box = {"panel": panel}
            mm_dt = f32r if USE_F32R else f32

            def emit_phaseB_chunk(j):
                panel = panel_box["panel"]
                rnd_i, owner = j // 8, j % 8
                gat = gathered[rnd_i]
                bt = bounces[j % 2][:].tensor
                # write banded rows (E rows at c*RS, O rows at c*RS-1; the
                # O guard slots j<3 are zero so the -1 spill is harmless).
                # Bounce writes go on the gpsimd queue: they depend on the
                # AllGather anyway, and gpsimd's FIFO is blocked by the
                # collective trigger until it completes.
                for i, nm in enumerate(names):
                    rowbase = {"E0": 0, "E1": 256, "O0": 1, "O1": 257}[nm]
                    odd = -1 if nm[0] == "O" else 0
                    nc.gpsimd.dma_start(
                        bass.AP(bt, rowbase * RS + odd,
                                [[2 * RS, 128], [PL, 2], [1, BW]]),
                        gat[owner, i].rearrange("p (r w) -> p r w", r=2))
                # skew reads into dense natural-row tiles (re|im planes)
                dense = {}
                for bblk in range(4):
                    dt_ = densep.tile([128, 2 * WIN], f32, tag=f"D{bblk}",
                                      name=f"dense{j}_{bblk}")
                    nc.sync.dma_start(
                        dt_[:].rearrange("p (r w) -> p r w", r=2).bitcast(f32r),
                        bass.AP(bt, 128 * bblk * (RS - 1) + WSTART[bblk] + PADB,
                                [[RS - 1, 128], [PL, 2], [1, WIN]]).bitcast(f32r))
                    dense[bblk] = dt_

                # panel apply
                psums = {}
                for m in range(4):
                    ps = psump2.tile([128, 2 * PCOLS], f32, tag=f"ps{m % 2}",
                                     name=f"psum{j}_{m}")
                    psums[m] = ps

                def mm(ps_ap, lhsT_ap, rhs_ap, start, stop=False):
                    nc.tensor.matmul(ps_ap, lhsT_ap.bitcast(mm_dt),
                                     rhs_ap.bitcast(mm_dt),
                                     start=start, stop=stop)

                for m in range(4):
                    mms = []
                    for b_ in (m, m - 1, m + 1):
                        if not 0 <= b_ < 4:
                            continue
                        moff = 128 * m - WSTART[b_]
                        mms.append((dense[b_][:, moff:moff + 128],
                                    panel[b_][:, PCOLS:3 * PCOLS]))
                        mms.append((dense[b_][:, WIN + moff:WIN + moff + 128],
                                    panel[b_][:, 0:2 * PCOLS]))
                    for i_, (lh, rh) in enumerate(mms):
                        mm(psums[m][:], lh, rh, start=(i_ == 0),
                           stop=(i_ == len(mms) - 1))
                newp = {}
                for m in range(4):
                    np_ = panelp.tile([128, 3 * PCOLS], f32, tag=f"P{m}",
                                      name=f"panel{j}_{m}")
                    # [re|im] <- psum ; imneg <- -psum_im (both round to f32r)
                    if m % 2 == 0:
                        nc.vector.tensor_copy(
                            np_[:, PCOLS:3 * PCOLS].bitcast(f32r), psums[m][:])
                        nc.scalar.mul(
                            np_[:, 0:PCOLS].bitcast(f32r),
                            psums[m][:, PCOLS:2 * PCOLS], -1.0)
                    else:
                        nc.scalar.mul(
                            np_[:, PCOLS:3 * PCOLS].bitcast(f32r),
                            psums[m][:], 1.0)
                        nc.vector.tensor_scalar_mul(
                            np_[:, 0:PCOLS].bitcast(f32r),
                            psums[m][:, PCOLS:2 * PCOLS], -1.0)
                    newp[m] = np_
                panel_box["panel"] = newp

            # ---------- phase A chunks with interleaved phase B ----------
            lay = 0
            for ch in range(CPC):
                # fresh leaf state
                cur = {}
                for nm in names:
                    t = leafp.tile([128, TW], f32, tag=f"L{nm}",
                                   name=f"c{ch}_{nm}")
                    nc.vector.memset(t[:], 0.0)
                    diag = 2 + BW + (PADB if nm[0] == "E" else PADB + 1)
                    nc.vector.memset(t[:, diag:diag + 1], 1.0)
                    cur[nm] = t

                for t_step in range(SCH):
                    lo_l = max(LO, PADB - 2 * t_step - 4)
                    hi_l = min(HI, PADB + 2 * t_step + 6)
                    # ---- H-type layer ----
                    E0, E1, O0, O1 = (cur[n] for n in names)
                    # shifted operands via PE (shift matrices, fp32):
                    # ep[k] = E[k+1], om[j] = O[j-1]; slot offsets applied
                    # at the consuming AP (-2 for ep, +2 for om).
                    sh = {}
                    for nm in ("ep0", "ep1", "om0", "om1"):
                        sh[nm] = psump.tile([128, TW], f32, tag=nm,
                                            name=f"sh{ch}_{t_step}_{nm}")
                    nc.tensor.matmul(sh["ep0"][:], SH_UP, E0[:], start=True,
                                     stop=False)
                    nc.tensor.matmul(sh["ep1"][:], SH_UP, E1[:], start=True,
                                     stop=True)
                    nc.tensor.matmul(sh["ep0"][:], SH_SELA, E1[:], start=False,
                                     stop=True)
                    nc.tensor.matmul(sh["om0"][:], SH_DN, O0[:], start=True,
                                     stop=True)
                    nc.tensor.matmul(sh["om1"][:], SH_DN, O1[:], start=True,
                                     stop=False)
                    nc.tensor.matmul(sh["om1"][:], SH_SELB, O0[:], start=False,
                                     stop=True)

                    new = {nm: leafp.tile([128, TW], f32, tag=f"L{nm}",
                                          name=f"h{ch}_{t_step}_{nm}")
                           for nm in names}
                    if ch == 0 and t_step == 0:
                        # one-time pad zeroing of this slot generation; the
                        # other slot is zeroed by the cur memsets above.
                        for nm in names:
                            nc.vector.memset(new[nm][:, 0:2], 0.0)
                            nc.vector.memset(new[nm][:, 2 + 3 * BW:TW], 0.0)
                    for b in range(PB):
                        E, O = (E0, O0) if b == 0 else (E1, O1)
                        nE, nO = (new["E0"], new["O0"]) if b == 0 else (new["E1"], new["O1"])
                        ep = sh["ep0"] if b == 0 else sh["ep1"]
                        om = sh["om0"] if b == 0 else sh["om1"]
                        emit_dst(lay, b, nO, O, ep, 0, lo_l, hi_l, off2=-2)
                        emit_dst(lay, b, nE, E, om, 4, lo_l, hi_l, off2=2)
                    cur = new
                    lay += 1

                    # ---- G-type layer ----
                    E0, E1, O0, O1 = (cur[n] for n in names)
                    new = {nm: leafp.tile([128, TW], f32, tag=f"L{nm}",
                                          name=f"g{ch}_{t_step}_{nm}")
                           for nm in names}
                    for b in range(PB):
                        E, O = (E0, O0) if b == 0 else (E1, O1)
                        nE, nO = (new["E0"], new["O0"]) if b == 0 else (new["E1"], new["O1"])
                        emit_dst(lay, b, nE, E, O, 0, lo_l, hi_l)
                        emit_dst(lay, b, nO, E, O, 4, lo_l, hi_l)
                    cur = new
                    lay += 1

                    # interleave one phase-B chunk of the previous round per
                    # phase-A step, so every engine FIFO alternates A/B work
                    # at fine grain and the collective overlaps compute.
                    if ch >= 1:
                        emit_phaseB_chunk(8 * (ch - 1) + t_step)

                # round leaves to f32r and send to DRAM + AllGather round
                send = dramp.tile([4, 128, 2 * BW], f32, name=f"send{ch}",
                                  tag=f"send{ch}")
                for i, nm in enumerate(names):
                    rnd = leafp.tile([128, 2 * BW], f32, tag=f"R{nm}",
                                     name=f"r{ch}_{nm}")
                    nc.vector.tensor_copy(rnd[:].bitcast(f32r),
                                          cur[nm][:, 2 + BW:2 + 3 * BW])
                    nc.sync.dma_start(send[i], rnd[:])
                gat = dramp.tile([NCORES, 4, 128, 2 * BW], f32,
                                 name=f"gat{ch}", tag=f"gat{ch}",
                                 addr_space="Shared")
                nc.gpsimd.collective_compute(
                    "AllGather", mybir.AluOpType.bypass,
                    replica_groups=[list(range(NCORES))],
                    ins=[send.opt()], outs=[gat.opt()],
                )
                gathered.append(gat)

            # last round of phase B after all phase A work
            for t_step in range(SCH):
                emit_phaseB_chunk(8 * (CPC - 1) + t_step)

            panel = panel_box["panel"]
            for b in range(4):
                nc.sync.dma_start(out_d.ap()[b], panel[b][:, PCOLS:3 * PCOLS])

    nc.compile()
    return nc


# ----------------------------------------------------------------------------
# Entry point
# ----------------------------------------------------------------------------

def assemble_output_v2(per_core):
    """per_core: list (cores 0..3 used) of [4, 128, 2*PCOLS] -> [N,N] c64."""
    M = np.zeros((N, N), np.complex64)
    for c in range(4):
        arr = per_core[c]
        cols = slice(c * PCOLS, (c + 1) * PCOLS)
        for b in range(4):
            rows = slice(b * 128, (b + 1) * 128)
            M[rows, cols] = arr[b, :, 0:PCOLS] + 1j * arr[b, :, PCOLS:2 * PCOLS]
    return M


_CACHE = {}


def kernel(**inputs) -> np.ndarray:
    import os

    from concourse.bass_utils import run_bass_kernel_spmd

    folded = host_fold_layers(inputs)
    if "nc" not in _CACHE:
        _CACHE["nc"] = build_program_v2()
    nc = _CACHE["nc"]

    shm = host_shift_mats()
    in_maps = [
        {"coef": host_coeff_core(inputs, c, folded),
         "pinit": host_panel_init(c), "shmats": shm}
        for c in range(NCORES)
    ]
    trace = bool(os.environ.get("KERNEL_TRACE"))
    res = run_bass_kernel_spmd(nc, in_maps, core_ids=list(range(NCORES)),
                               trace=trace)
    if res.exec_time_ns is not None:
        print(f"HW exec time: {res.exec_time_ns} ns")
    return assemble_output_v2([r["mout"] for r in res.results])



# revision 21
# speedup vs baseline: 1.0959x; 1.0290x over previous
"""Trainium2 Bass kernel v2 for the Clements mesh chain (N=512).

Strategy (two-phase, chunk-reassociated):
  Host folds the 1538 primitive layers into 512 2x2-block complex layers
  (G on even pairs, H on odd pairs), then splits the 256 steps into 32
  chunks of 8 steps.

  Phase A (parallel across cores): core c builds the TRANSPOSED chunk
  products W_j.T for j in {c, 8+c, 16+c, 24+c} by running the reversed
  chain on a banded (diagonal-offset) representation with per-partition
  scalar fused ops (tensor_scalar + scalar_tensor_tensor, DVE) in a
  pair-partition layout.  Band half-width <= 17, so per-layer ops are
  [128, ~36] instead of [128, 512].

  Leaves are AllGather'd in 4 rounds of 8 (overlapped with later chunks).

  Phase B (duplicated panel): every core expands each banded leaf to
  dense natural-row lhsT tiles via a skewed DRAM bounce (the skew
  absorbs both the pair->natural row permutation and the diag-offset ->
  absolute-column conversion), then applies the 32 chunk matrices
  sequentially to a 128-column identity panel with PE matmuls
  (fp32r, N=256, ~20 matmuls/chunk).  Cores c and c+4 duplicate the
  same 128-column group; cores 0-3's outputs are used.
"""

import numpy as np

N = 512
S = 256
NCORES = 8
NCH = 32             # chunks
SCH = S // NCH       # 8 steps per chunk
CPC = NCH // NCORES  # 4 chunks per core
LAY = 2 * SCH        # 16 layers emitted per chunk (H.T, G.T per reversed step)
NLAY = CPC * LAY     # 64 layers per core
PB = 2
NSLOT = 8
BW = 40              # banded slots per plane
TW = 3 * BW + 4      # state tile width: [pad2 | imneg | re | im | pad2]
PADB = 19            # slot of diagonal (pair-base column offset 0)
RS = 416             # skew bounce row stride (>= ~404)
WIN = 384            # dense window per natural K-block (3 full blocks)
PCOLS = 64           # panel columns per core (8 distinct column groups)
USE_F32R = True

# dense window starts per K-block b: 3 full M-blocks so every matmul is
# a full M=128 (PE only allows nice dst partition bases; M never affects
# matmul time, which is ~N cycles)
WSTART = [0, 0, 128, 128]
# nonzero stripe of each K-block's dense window (m rel to WSTART):
# E rows contribute m-K+WSTART in [-17,19), O rows (stored shifted -1)
# in [-18,18); union [-18,19) over K in [128b,128b+128), clip to [0,WIN)
SUBLO = [0, 109, 109, 237]
SUBHI = [148, 276, 276, 384]


# ----------------------------------------------------------------------------
# Host math (same folding as v1)
# ----------------------------------------------------------------------------

def _mmi_2x2(loss, imb):
    a = np.sqrt(1.0 - loss.astype(np.float64))
    t = a * np.sqrt(0.5 + imb.astype(np.float64))
    r = a * np.sqrt(0.5 - imb.astype(np.float64))
    m = np.zeros(loss.shape + (2, 2), np.complex128)
    m[..., 0, 0] = t
    m[..., 1, 1] = t
    m[..., 0, 1] = 1j * r
    m[..., 1, 0] = 1j * r
    return m


def _pc_vec(theta, loss):
    return np.sqrt(1.0 - loss.astype(np.float64)) * np.exp(1j * theta.astype(np.float64))


def host_fold_layers(inputs):
    th = np.asarray(inputs["thetas_full"], np.float64)
    lp = np.asarray(inputs["pc_losses_full"], np.float64)
    tio = np.asarray(inputs["thetas_inout"], np.float64)
    lio = np.asarray(inputs["pc_losses_inout"], np.float64)
    le = np.asarray(inputs["mmi_losses_even"], np.float64)
    ie = np.asarray(inputs["mmi_imb_even"], np.float64)
    lo = np.asarray(inputs["mmi_losses_odd"], np.float64)
    io = np.asarray(inputs["mmi_imb_odd"], np.float64)

    G = np.zeros((S, 256, 2, 2), np.complex128)
    Hp = np.zeros((S, 255, 2, 2), np.complex128)
    h_edge = np.zeros((S, 2), np.complex128)

    for s in range(S):
        e1 = _mmi_2x2(le[2 * s], ie[2 * s])
        e2 = _mmi_2x2(le[2 * s + 1], ie[2 * s + 1])
        a0 = _pc_vec(th[2 * s], lp[2 * s]).reshape(256, 2)
        G[s] = e2 @ (a0[:, :, None] * e1)

        o1 = _mmi_2x2(lo[2 * s], io[2 * s])
        o2 = _mmi_2x2(lo[2 * s + 1], io[2 * s + 1])
        a1 = _pc_vec(th[2 * s + 1], lp[2 * s + 1])
        a1p = a1[1:-1].reshape(255, 2)
        Hp[s] = o2 @ (a1p[:, :, None] * o1)
        h_edge[s, 0] = a1[0]
        h_edge[s, 1] = a1[-1]

    ain = _pc_vec(tio[0], lio[0]).reshape(256, 2)
    G[0] = G[0] * ain[:, None, :]
    aout = _pc_vec(tio[1], lio[1])
    Hp[S - 1] = aout[1:-1].reshape(255, 2)[:, :, None] * Hp[S - 1]
    h_edge[S - 1, 0] *= aout[0]
    h_edge[S - 1, 1] *= aout[-1]
    return G, Hp, h_edge


def _h_coeffs(Hp_s, edge_s, transpose):
    """One H layer -> (c00, c01, d10, d11) arrays [256] for the E/O update.

    Matrix form: rows 2k+1/2k+2 have block [[h00, h01],[h10, h11]] = Hp[k];
    rows 0 and 511 have scalars edge[0], edge[1].
    Transposed layer swaps h01/h10.
    """
    h00, h01 = Hp_s[:, 0, 0], Hp_s[:, 0, 1]
    h10, h11 = Hp_s[:, 1, 0], Hp_s[:, 1, 1]
    if transpose:
        h01, h10 = h10, h01
    c00 = np.zeros(256, np.complex128)
    c01 = np.zeros(256, np.complex128)
    d10 = np.zeros(256, np.complex128)
    d11 = np.zeros(256, np.complex128)
    c00[:255] = h00
    c00[255] = edge_s[1]
    c01[:255] = h01
    d11[1:] = h11
    d11[0] = edge_s[0]
    d10[1:] = h10
    return c00, c01, d10, d11


def _plane8_G(g):
    """g: [256,2,2] complex -> [256,8] f32 chain scalars (imneg scheme).

    dst nE = g00*E + g01*O ; dst nO = g10*E + g11*O.  Each complex coeff c
    contributes two per-partition scalars: c.real on view [re|im] and
    c.imag on view [-im|re].
    """
    g00, g01 = g[:, 0, 0], g[:, 0, 1]
    g10, g11 = g[:, 1, 0], g[:, 1, 1]
    planes = [
        g00.real, g00.imag, g01.real, g01.imag,
        g10.real, g10.imag, g11.real, g11.imag,
    ]
    return np.stack(planes, axis=-1).astype(np.float32)


def _plane8_H(c00, c01, d10, d11):
    """dst nO = c00*O + c01*ep ; dst nE = d11*E + d10*om."""
    planes = [
        c00.real, c00.imag, c01.real, c01.imag,
        d11.real, d11.imag, d10.real, d10.imag,
    ]
    return np.stack(planes, axis=-1).astype(np.float32)


def host_coeff_core(inputs, core, folded=None):
    """Per-core coefficient array [128, NLAY*PB*NSLOT].

    Core c builds W_j.T for chunks j = c, 8+c, 16+c, 24+c.  W_j.T is the
    product of transposed layers in reversed order: for s from high to
    low, apply H_s.T then G_s.T.
    """
    G, Hp, h_edge = folded if folded is not None else host_fold_layers(inputs)
    layers = []   # [256, 16] plane stacks in emission order
    for ch in range(CPC):
        j = 8 * ch + core
        s0, s1 = j * SCH, (j + 1) * SCH
        for s in range(s1 - 1, s0 - 1, -1):
            c00, c01, d10, d11 = _h_coeffs(Hp[s], h_edge[s], transpose=True)
            layers.append(_plane8_H(c00, c01, d10, d11))
            gT = G[s].transpose(0, 2, 1)
            layers.append(_plane8_G(gT))
    arr = np.stack(layers, axis=0)                    # [NLAY, 256, 8]
    arr = arr.reshape(NLAY, PB, 128, NSLOT).transpose(2, 0, 1, 3)
    return np.ascontiguousarray(arr.reshape(128, NLAY * PB * NSLOT))


def host_shift_mats():
    """Constant PE shift matrices [4,128,128]: lhsT[k,m] nonzero -> out[m]=in[k]."""
    m = np.zeros((4, 128, 128), np.float32)
    for i in range(127):
        m[0, i + 1, i] = 1.0      # SH_UP: out[m] = in[m+1]
        m[2, i, i + 1] = 1.0      # SH_DN: out[m] = in[m-1]
    m[1, 0, 127] = 1.0            # SELA: out[127] = in[0]
    m[3, 127, 0] = 1.0            # SELB: out[0] = in[127]
    # SBUF layout [128 partitions, 4*128]: partition = k (contraction)
    return np.ascontiguousarray(m.transpose(1, 0, 2).reshape(128, 512))


def host_panel_init(core):
    """Identity panel [4, 128, 3*PCOLS] (imneg|re|im), natural rows."""
    arr = np.zeros((4, 128, 3 * PCOLS), np.float32)
    colbase = core * PCOLS
    for j in range(PCOLS):
        row = colbase + j
        b, p = divmod(row, 128)
        arr[b, p, PCOLS + j] = 1.0   # re plane
    return arr


# ----------------------------------------------------------------------------
# Device program
# ----------------------------------------------------------------------------

def build_program_v2():
    import concourse.bass as bass
    import concourse.tile as tile
    from concourse import bacc, mybir

    f32 = mybir.dt.float32
    f32r = mybir.dt.float32r
    MUL = mybir.AluOpType.mult
    ADD = mybir.AluOpType.add

    nc = bacc.Bacc("TRN2", target_bir_lowering=False, debug=False,
                   num_devices=NCORES)
    coef_d = nc.dram_tensor("coef", [128, NLAY * PB * NSLOT], f32,
                            kind="ExternalInput")
    pinit_d = nc.dram_tensor("pinit", [4, 128, 3 * PCOLS], f32,
                             kind="ExternalInput")
    shm_d = nc.dram_tensor("shmats", [128, 4 * 128], f32, kind="ExternalInput")
    out_d = nc.dram_tensor("mout", [4, 128, 2 * PCOLS], f32,
                           kind="ExternalOutput")

    names = ("E0", "E1", "O0", "O1")

    with tile.TileContext(nc) as tc:
        with (
            tc.tile_pool(name="coefp", bufs=1) as coefp,
            tc.tile_pool(name="leafp", bufs=2) as leafp,
            tc.tile_pool(name="shiftp", bufs=2) as shiftp,
            tc.tile_pool(name="densep", bufs=3) as densep,
            tc.tile_pool(name="panelp", bufs=2) as panelp,
            tc.tile_pool(name="psump", bufs=1, space="PSUM") as psump,
            tc.tile_pool(name="psump2", bufs=2, space="PSUM") as psump2,
            tc.tile_pool(name="dramp", bufs=1, space="DRAM") as dramp,
            tc.tile_pool(name="bouncep", bufs=1, space="DRAM") as bouncep,
        ):
            coef = coefp.tile([128, NLAY * PB * NSLOT], f32)
            nc.sync.dma_start(coef[:], coef_d.ap())
            zeros = coefp.tile([128, RS], f32)
            nc.vector.memset(zeros[:], 0.0)
            shm = coefp.tile([128, 4 * 128], f32)
            nc.sync.dma_start(shm[:], shm_d.ap())
            SH_UP, SH_SELA, SH_DN, SH_SELB = (
                shm[:, 128 * i:128 * (i + 1)] for i in range(4))

            panel = {}
            for b in range(4):
                traw = coefp.tile([128, 3 * PCOLS], f32, name=f"pinit_raw_{b}")
                nc.sync.dma_start(traw[:], pinit_d.ap()[b])
                t = panelp.tile([128, 3 * PCOLS], f32, tag=f"P{b}",
                                name=f"panel_init_{b}")
                nc.vector.tensor_copy(t[:].bitcast(f32r), traw[:])
                panel[b] = t

            def csc(lay, b, slot, p0=0, p1=128):
                idx = (lay * PB + b) * NSLOT + slot
                return coef[p0:p1, idx:idx + 1]

            # ---------- phase A helpers (banded chain, imneg scheme) ----------
            # State tiles [128, TW]: [pad2 | imneg(BW) | re(BW) | im(BW) | pad2].
            # A complex MAC  dst += c*src  is two per-partition-scalar ops:
            #   dst[re|im] += c.real * src[re|im]  +  c.imag * src[-im|re]

            LO, HI = 2, BW - 2

            def p3(t):
                return t[:, 2:2 + 3 * BW].rearrange("p (a w) -> p a w", a=3)

            def v1(t, lo, hi, off=0):
                return p3(t)[:, 1:3, lo + off:hi + off]     # [re|im]

            def v2(t, lo, hi, off=0):
                return p3(t)[:, 0:2, lo + off:hi + off]     # [-im|re]

            def emit_dst(lay, b, nt, s1t, s2t, base, lo, hi, off2=0):
                """nt = c1*s1t + c2*s2t (complex per-partition coeffs) plus
                imneg-plane maintenance.  s2t may be a shifted PSUM tile
                consumed at slot offset off2.  The first op spans the full
                [re|im] planes so guard cols propagate zeros."""
                full = nt[:, 2 + BW:2 + 3 * BW]
                nc.vector.tensor_scalar_mul(full, s1t[:, 2 + BW:2 + 3 * BW],
                                            csc(lay, b, base + 0))
                d = v1(nt, lo, hi)
                nc.vector.scalar_tensor_tensor(
                    d, v2(s1t, lo, hi), csc(lay, b, base + 1), d, MUL, ADD)
                nc.vector.scalar_tensor_tensor(
                    d, v1(s2t, lo, hi, off2), csc(lay, b, base + 2), d, MUL, ADD)
                nc.vector.scalar_tensor_tensor(
                    d, v2(s2t, lo, hi, off2), csc(lay, b, base + 3), d, MUL, ADD)
                # imneg plane <- -im
                nc.scalar.mul(nt[:, 2:2 + BW],
                              nt[:, 2 + 2 * BW:2 + 3 * BW], -1.0)

            # ---------- phase B plumbing (emitted interleaved) ----------
            PL = 512 * RS   # plane stride in bounce
            bounces = []
            for bi in range(2):
                bo = bouncep.tile([2 * PL], f32, name=f"bounce{bi}")
                for pl in range(2):
                    for q in range(4):
                        nc.sync.dma_start(
                            bass.AP(bo[:].tensor, pl * PL + q * 128 * RS,
                                    [[RS, 128], [1, RS]]),
                            zeros[:])
                bounces.append(bo)

            gathered = []
            panel_box = {"panel": panel}
            mm_dt = f32r if USE_F32R else f32

            def emit_phaseB_chunk(j):
                panel = panel_box["panel"]
                rnd_i, owner = j // 8, j % 8
                gat = gathered[rnd_i]
                bt = bounces[j % 2][:].tensor
                # write banded rows (E rows at c*RS, O rows at c*RS-1; the
                # O guard slots j<3 are zero so the -1 spill is harmless).
                # Bounce writes go on the gpsimd queue: they depend on the
                # AllGather anyway, and gpsimd's FIFO is blocked by the
                # collective trigger until it completes.
                for i, nm in enumerate(names):
                    rowbase = {"E0": 0, "E1": 256, "O0": 1, "O1": 257}[nm]
                    odd = -1 if nm[0] == "O" else 0
                    nc.gpsimd.dma_start(
                        bass.AP(bt, rowbase * RS + odd,
                                [[2 * RS, 128], [PL, 2], [1, BW]]),
                        gat[owner, i].rearrange("p (r w) -> p r w", r=2))
                # skew reads of the nonzero stripe into pre-zeroed dense
                # tiles (fixed stripe position per K-block across chunks)
                tail = j >= 24
                dissue = [nc.scalar, nc.sync] if tail else [nc.sync, nc.sync]
                dense = {}
                for bblk in range(4):
                    dt_ = densep.tile([128, 2 * WIN], f32, tag=f"D{bblk}",
                                      name=f"dense{j}_{bblk}")
                    lo_b, w_b = 0, WIN
                    dst = dt_[:].rearrange("p (r w) -> p r w", r=2)
                    dissue[bblk % 2].dma_start(
                        dst[:, :, lo_b:lo_b + w_b].bitcast(f32r),
                        bass.AP(bt, 128 * bblk * (RS - 1) + WSTART[bblk]
                                + PADB + lo_b,
                                [[RS - 1, 128], [PL, 2], [1, w_b]]).bitcast(f32r))
                    dense[bblk] = dt_

                # panel apply
                psums = {}
                for m in range(4):
                    ps = psump2.tile([128, 2 * PCOLS], f32, tag=f"ps{m % 2}",
                                     name=f"psum{j}_{m}")
                    psums[m] = ps

                def psv(m):
                    return psums[m][:]

                def mm(ps_ap, lhsT_ap, rhs_ap, start, stop=False):
                    nc.tensor.matmul(ps_ap, lhsT_ap.bitcast(mm_dt),
                                     rhs_ap.bitcast(mm_dt),
                                     start=start, stop=stop)

                for m in range(4):
                    mms = []
                    for b_ in (m, m - 1, m + 1):
                        if not 0 <= b_ < 4:
                            continue
                        moff = 128 * m - WSTART[b_]
                        mms.append((dense[b_][:, moff:moff + 128],
                                    panel[b_][:, PCOLS:3 * PCOLS]))
                        mms.append((dense[b_][:, WIN + moff:WIN + moff + 128],
                                    panel[b_][:, 0:2 * PCOLS]))
                    for i_, (lh, rh) in enumerate(mms):
                        mm(psv(m), lh, rh, start=(i_ == 0),
                           stop=(i_ == len(mms) - 1))
                newp = {}
                for m in range(4):
                    np_ = panelp.tile([128, 3 * PCOLS], f32, tag=f"P{m}",
                                      name=f"panel{j}_{m}")
                    # [re|im] <- psum ; imneg <- -psum_im (both round to f32r)
                    if m % 2 == 0:
                        nc.vector.tensor_copy(
                            np_[:, PCOLS:3 * PCOLS].bitcast(f32r), psv(m))
                        nc.scalar.mul(
                            np_[:, 0:PCOLS].bitcast(f32r),
                            psv(m)[:, PCOLS:2 * PCOLS], -1.0)
                    else:
                        nc.scalar.mul(
                            np_[:, PCOLS:3 * PCOLS].bitcast(f32r), psv(m), 1.0)
                        nc.vector.tensor_scalar_mul(
                            np_[:, 0:PCOLS].bitcast(f32r),
                            psv(m)[:, PCOLS:2 * PCOLS], -1.0)
                    newp[m] = np_
                panel_box["panel"] = newp

            # ---------- phase A chunks with interleaved phase B ----------
            lay = 0
            for ch in range(CPC):
                # fresh leaf state
                cur = {}
                for nm in names:
                    t = leafp.tile([128, TW], f32, tag=f"L{nm}",
                                   name=f"c{ch}_{nm}")
                    nc.vector.memset(t[:], 0.0)
                    diag = 2 + BW + (PADB if nm[0] == "E" else PADB + 1)
                    nc.vector.memset(t[:, diag:diag + 1], 1.0)
                    cur[nm] = t

                for t_step in range(SCH):
                    lo_l = max(LO, PADB - 2 * t_step - 4)
                    hi_l = min(HI, PADB + 2 * t_step + 6)
                    # ---- H-type layer ----
                    E0, E1, O0, O1 = (cur[n] for n in names)
                    # shifted operands via PE (shift matrices, fp32):
                    # ep[k] = E[k+1], om[j] = O[j-1]; slot offsets applied
                    # at the consuming AP (-2 for ep, +2 for om).
                    sh = {}
                    for nm in ("ep0", "ep1", "om0", "om1"):
                        sh[nm] = psump.tile([128, TW], f32, tag=nm,
                                            name=f"sh{ch}_{t_step}_{nm}")
                    nc.tensor.matmul(sh["ep0"][:], SH_UP, E0[:], start=True,
                                     stop=False)
                    nc.tensor.matmul(sh["ep1"][:], SH_UP, E1[:], start=True,
                                     stop=True)
                    nc.tensor.matmul(sh["ep0"][:], SH_SELA, E1[:], start=False,
                                     stop=True)
                    nc.tensor.matmul(sh["om0"][:], SH_DN, O0[:], start=True,
                                     stop=True)
                    nc.tensor.matmul(sh["om1"][:], SH_DN, O1[:], start=True,
                                     stop=False)
                    nc.tensor.matmul(sh["om1"][:], SH_SELB, O0[:], start=False,
                                     stop=True)

                    new = {nm: leafp.tile([128, TW], f32, tag=f"L{nm}",
                                          name=f"h{ch}_{t_step}_{nm}")
                           for nm in names}
                    if ch == 0 and t_step == 0:
                        # one-time pad zeroing of this slot generation; the
                        # other slot is zeroed by the cur memsets above.
                        for nm in names:
                            nc.vector.memset(new[nm][:, 0:2], 0.0)
                            nc.vector.memset(new[nm][:, 2 + 3 * BW:TW], 0.0)
                    for b in range(PB):
                        E, O = (E0, O0) if b == 0 else (E1, O1)
                        nE, nO = (new["E0"], new["O0"]) if b == 0 else (new["E1"], new["O1"])
                        ep = sh["ep0"] if b == 0 else sh["ep1"]
                        om = sh["om0"] if b == 0 else sh["om1"]
                        emit_dst(lay, b, nO, O, ep, 0, lo_l, hi_l, off2=-2)
                        emit_dst(lay, b, nE, E, om, 4, lo_l, hi_l, off2=2)
                    cur = new
                    lay += 1

                    # ---- G-type layer ----
                    E0, E1, O0, O1 = (cur[n] for n in names)
                    new = {nm: leafp.tile([128, TW], f32, tag=f"L{nm}",
                                          name=f"g{ch}_{t_step}_{nm}")
                           for nm in names}
                    for b in range(PB):
                        E, O = (E0, O0) if b == 0 else (E1, O1)
                        nE, nO = (new["E0"], new["O0"]) if b == 0 else (new["E1"], new["O1"])
                        emit_dst(lay, b, nE, E, O, 0, lo_l, hi_l)
                        emit_dst(lay, b, nO, E, O, 4, lo_l, hi_l)
                    cur = new
                    lay += 1

                    # interleave one phase-B chunk of the previous round per
                    # phase-A step, so every engine FIFO alternates A/B work
                    # at fine grain and the collective overlaps compute.
                    if ch >= 1:
                        emit_phaseB_chunk(8 * (ch - 1) + t_step)

                # round leaves to f32r and send to DRAM + AllGather round
                send = dramp.tile([4, 128, 2 * BW], f32, name=f"send{ch}",
                                  tag=f"send{ch}")
                for i, nm in enumerate(names):
                    rnd = leafp.tile([128, 2 * BW], f32, tag=f"R{nm}",
                                     name=f"r{ch}_{nm}")
                    nc.vector.tensor_copy(rnd[:].bitcast(f32r),
                                          cur[nm][:, 2 + BW:2 + 3 * BW])
                    nc.sync.dma_start(send[i], rnd[:])
                gat = dramp.tile([NCORES, 4, 128, 2 * BW], f32,
                                 name=f"gat{ch}", tag=f"gat{ch}",
                                 addr_space="Shared")
                nc.gpsimd.collective_compute(
                    "AllGather", mybir.AluOpType.bypass,
                    replica_groups=[list(range(NCORES))],
                    ins=[send.opt()], outs=[gat.opt()],
                )
                gathered.append(gat)

            # last round of phase B after all phase A work
            for t_step in range(SCH):
                emit_phaseB_chunk(8 * (CPC - 1) + t_step)

            panel = panel_box["panel"]
            for b in range(4):
                nc.sync.dma_start(out_d.ap()[b], panel[b][:, PCOLS:3 * PCOLS])

    nc.compile()
    return nc


# ----------------------------------------------------------------------------
# Entry point
# ----------------------------------------------------------------------------

def assemble_output_v2(per_core):
    """per_core: list (all 8 cores) of [4, 128, 2*PCOLS] -> [N,N] c64."""
    M = np.zeros((N, N), np.complex64)
    for c in range(NCORES):
        arr = per_core[c]
        cols = slice(c * PCOLS, (c + 1) * PCOLS)
        for b in range(4):
            rows = slice(b * 128, (b + 1) * 128)
            M[rows, cols] = arr[b, :, 0:PCOLS] + 1j * arr[b, :, PCOLS:2 * PCOLS]
    return M


_CACHE = {}


def kernel(**inputs) -> np.ndarray:
    import os

    from concourse.bass_utils import run_bass_kernel_spmd

    folded = host_fold_layers(inputs)
    if "nc" not in _CACHE:
        _CACHE["nc"] = build_program_v2()
    nc = _CACHE["nc"]

    shm = host_shift_mats()
    in_maps = [
        {"coef": host_coeff_core(inputs, c, folded),
         "pinit": host_panel_init(c), "shmats": shm}
        for c in range(NCORES)
    ]
    trace = bool(os.environ.get("KERNEL_TRACE"))
    res = run_bass_kernel_spmd(nc, in_maps, core_ids=list(range(NCORES)),
                               trace=trace)
    if res.exec_time_ns is not None:
        print(f"HW exec time: {res.exec_time_ns} ns")
    return assemble_output_v2([r["mout"] for r in res.results])

